# revision 17
# baseline (speedup 1.0000x reference)
"""Gaussian overlap loss (pairwise Bhattacharyya coefficients) on 8 TRN2 cores.

Math: for 2x2 SPD sigma_i = [[a,b],[b,c]], det_s = ac-b^2, r = sqrt(det_s):
  quad_ij = (cM dx^2 - 2 bM dx dy + aM dy^2) / detM   (M = pairwise average)
  coef_ij = exp(-quad/8) * sqrt(sqrt(det_s_i det_s_j) / detM)
          = exp(-0.5 * (ln D' + 0.25 * N'/D'))
where N' = quad_numerator/(r_i r_j) (rank-16 bilinear form; the 0.25 factor
is folded into the f-side features on host) and D' = detM/(r_i r_j) >= 1
(rank-5 bilinear form, fp16).  N' needs ~18 significand bits (huge
cancelling xc^2-scale terms), done as a bf16 hi/lo split via three
PSUM-accumulated matmuls fh.gh + fh.gl + fl.gh.

Per-element loss = alpha*coef + (1-alpha)*relu(coef-beta); masked
(close_mask/diagonal) pairs contribute 0, so with w_ij = 2 - cm_ij - cm_ji:
  S = alpha*A + (1-alpha)*B
  A = 2*sum_{i<j} coef          - sum_{i<j}(cm_ij+cm_ji) coef
  B = 2*sum_{i<j} relu(coef-b)  - sum_{i<j}(cm_ij+cm_ji) relu(coef-b)
The device computes the dense upper-triangle sums (A via the Exp
activation's accum_out; B via sum of max(coef,beta) minus beta*count); the
1%-dense close-mask corrections and the eigenvalue regularizer are sparse
fp64 sums on host.

Pair pruning: points sorted by x, 32 chunks of 128. Rigorous chunk-pair
bound max coef <= exp(-0.125 d(bbox)^2 / lam_max) drops far pairs within an
error budget. Each core runs T tiles of [128,512]:
 - Te "extra" tiles: 4 gathered partner chunks vs one row chunk (pad
   columns get gh=1e20 so coef==0 exactly); B-relevant extras packed first.
 - 4 diagonal-window tiles (consecutive own chunks 4k..4k+3): G data comes
   from one per-core column strip [*, 896] sliced per tile; the j>i
   triangle is enforced by gpsimd.affine_select(fill=+1e30) on the exp arg.

DMA is the wall (~22 GB/s per core, independent of queue count) and each
dma_start costs ~0.6us on its issuing engine, so tiles ship as TWO packed
DMAs (F [21,256], G [21,1024]; fp16 D-features ride rows 16-20 as raw
bits) and all DMAs issue from the sync engine.
"""

import numpy as np
import ml_dtypes

import concourse.bacc as bacc
import concourse.tile as tile
from concourse import mybir
from concourse.bass_utils import run_bass_kernel_spmd

N = 4096
CH = 128
NCH = N // CH
TILE_F = 512
N_CORES = 8
LAMB = 1e-4
ALPHA = 0.01
BETA = 0.6065
EPS = 1e-7
T_A = 2e-4
PAD_GH = 1.0e20
PAD_GD = 4.0e4

f32 = mybir.dt.float32
bf16 = mybir.dt.bfloat16
fp16 = mybir.dt.float16

_orig_get_activation_tables = bacc.get_activation_tables


def _pinned_activation_tables(module_arch):
    tables = _orig_get_activation_tables(module_arch)
    pin = {mybir.ActivationFunctionType.Exp, mybir.ActivationFunctionType.Ln}
    shared = "natural_log_exp_and_others"
    if shared in tables and pin <= tables[shared]:
        tables = {name: (fns if name == shared else fns - pin)
                  for name, fns in tables.items()}
    return tables


bacc.get_activation_tables = _pinned_activation_tables

_BUILD_CACHE = {}


def build_kernel(T, Te, TBe):
    """T total tiles; Te extra tiles (first), 4 diag tiles last; B-op on
    extras t < TBe and on all diag tiles."""
    key = (T, Te, TBe)
    if key in _BUILD_CACHE:
        return _BUILD_CACHE[key]
    AF = mybir.ActivationFunctionType
    ALU = mybir.AluOpType

    nc = bacc.Bacc("TRN2", target_bir_lowering=False, debug=False,
                   num_devices=N_CORES)
    TeX = max(Te, 1)
    ft_d = nc.dram_tensor("ft", [16, TeX * 256], bf16, kind="ExternalInput").ap()
    fd_d = nc.dram_tensor("fd", [5, TeX * 128], fp16, kind="ExternalInput").ap()
    gt_d = nc.dram_tensor("gt", [16, TeX * 1024], bf16, kind="ExternalInput").ap()
    gd_d = nc.dram_tensor("gd", [5, TeX * 512], fp16, kind="ExternalInput").ap()
    fs_d = nc.dram_tensor("fs", [16, 1024], bf16, kind="ExternalInput").ap()
    fds_d = nc.dram_tensor("fds", [5, 512], fp16, kind="ExternalInput").ap()
    gs_d = nc.dram_tensor("gs", [16, 1792], bf16, kind="ExternalInput").ap()
    gds_d = nc.dram_tensor("gds", [5, 896], fp16, kind="ExternalInput").ap()
    out = nc.dram_tensor("out", [CH, 2 * T], f32, kind="ExternalOutput").ap()

    with tile.TileContext(nc) as tc:
        with (
            tc.tile_pool(name="consts", bufs=1) as consts,
            tc.tile_pool(name="strip", bufs=1) as strip,
            tc.tile_pool(name="fio", bufs=2) as fio,
            tc.tile_pool(name="gio", bufs=2) as gio,
            tc.tile_pool(name="work", bufs=3) as work,
            tc.tile_pool(name="psum", bufs=4, space="PSUM") as psum,
        ):
            ones = consts.tile([CH, TILE_F], bf16)
            nc.vector.memset(ones[:], 1.0)
            sA = consts.tile([CH, T], f32)
            sB = consts.tile([CH, T], f32)
            nc.vector.memset(sB[:], 0.0)

            fs = strip.tile([16, 1024], bf16)
            fds = strip.tile([5, 512], fp16)
            gs = strip.tile([16, 1792], bf16)
            gds = strip.tile([5, 896], fp16)
            fta = strip.tile([16, TeX * 256], bf16)
            fda = strip.tile([5, TeX * 128], fp16)
            gta = strip.tile([16, TeX * 1024], bf16)
            gda = strip.tile([5, TeX * 512], fp16)

            # stage everything up front: first half of the extras' G first
            # (tile 0 compute can start ~1/4 into the DMA stream), then the
            # rest; ~10 large DMAs total (dma_start costs ~0.6us of engine
            # time each, so few-and-large beats per-tile streaming).
            half = (TeX * 1024) // 2
            nc.sync.dma_start(out=gta[:, 0:half], in_=gt_d[:, 0:half])
            nc.sync.dma_start(out=fta[:], in_=ft_d)
            nc.sync.dma_start(out=fda[:], in_=fd_d)
            nc.sync.dma_start(out=gda[:], in_=gd_d)
            nc.sync.dma_start(out=gta[:, half:], in_=gt_d[:, half:])
            nc.sync.dma_start(out=gs[:], in_=gs_d)
            nc.sync.dma_start(out=fs[:], in_=fs_d)
            nc.sync.dma_start(out=fds[:], in_=fds_d)
            nc.sync.dma_start(out=gds[:], in_=gds_d)

            def tile_body(t, fh, fl, fd, gh, gl, gd, diag):
                # pd first: rec/ln depend only on it, so they can start
                # while the three pn matmuls still accumulate
                pd = psum.tile([CH, TILE_F], f32, tag="pd")
                nc.tensor.matmul(pd[:], lhsT=fd, rhs=gd, start=True, stop=True)
                pn = psum.tile([CH, TILE_F], f32, tag="pn")
                nc.tensor.matmul(pn[:], lhsT=fh, rhs=gh, start=True, stop=False)
                nc.tensor.matmul(pn[:], lhsT=fh, rhs=gl, start=False, stop=False)
                nc.tensor.matmul(pn[:], lhsT=fl, rhs=gh, start=False, stop=True)

                rec = work.tile([CH, TILE_F], f32, tag="rec")
                nc.vector.reciprocal_approx_fast(out=rec[:], in_=pd[:])
                lD = work.tile([CH, TILE_F], bf16, tag="lD")
                nc.scalar.activation(lD[:], pd[:], AF.Ln)
                mq = work.tile([CH, TILE_F], f32, tag="mq")
                nc.vector.tensor_tensor(mq[:], pn[:], rec[:], ALU.mult)
                tq = work.tile([CH, TILE_F], f32, tag="tq")
                nc.gpsimd.tensor_tensor(tq[:], lD[:], mq[:], ALU.add)
                if diag:
                    # strictly-upper mask only matters in the self-chunk
                    # (first 128 cols); done in place on that strip
                    nc.gpsimd.affine_select(
                        tq[:, 0:CH], tq[:, 0:CH], pattern=[[1, CH]],
                        compare_op=ALU.is_gt, fill=1e30,
                        base=0, channel_multiplier=-1)
                c0 = work.tile([CH, TILE_F], bf16, tag="c0")
                nc.scalar.activation(c0[:], tq[:], AF.Exp, scale=-0.5,
                                     accum_out=sA[:, t:t + 1])
                if diag or t < TBe:
                    scr = work.tile([CH, TILE_F], bf16, tag="scr")
                    nc.vector.scalar_tensor_tensor(
                        out=scr[:], in0=c0[:], scalar=BETA, in1=ones[:],
                        op0=ALU.max, op1=ALU.mult,
                        accum_out=sB[:, t:t + 1])

            def extra_tile(t):
                tile_body(
                    t,
                    fh=fta[:, t * 256:t * 256 + 128],
                    fl=fta[:, t * 256 + 128:(t + 1) * 256],
                    fd=fda[:, t * 128:(t + 1) * 128],
                    gh=gta[:, t * 1024:t * 1024 + 512],
                    gl=gta[:, t * 1024 + 512:(t + 1) * 1024],
                    gd=gda[:, t * 512:(t + 1) * 512],
                    diag=False)

            def diag_tile(j):
                tile_body(
                    Te + j,
                    fh=fs[0:16, j * 128:(j + 1) * 128],
                    fl=fs[0:16, 512 + j * 128:512 + (j + 1) * 128],
                    fd=fds[0:5, j * 128:(j + 1) * 128],
                    gh=gs[0:16, j * 128:j * 128 + 512],
                    gl=gs[0:16, 896 + j * 128:896 + j * 128 + 512],
                    gd=gds[0:5, j * 128:j * 128 + 512],
                    diag=True)

            for t in range(Te):
                extra_tile(t)
            for j in range(4):
                diag_tile(j)

            nc.sync.dma_start(out=out[:, 0:T], in_=sA[:])
            nc.sync.dma_start(out=out[:, T:2 * T], in_=sB[:])

    nc.compile()
    _BUILD_CACHE[key] = nc
    return nc


def _features(mu, sigma):
    fp = np.float32
    a = sigma[:, 0, 0].astype(fp)
    b = sigma[:, 0, 1].astype(fp)
    c = sigma[:, 1, 1].astype(fp)
    x = mu[:, 0].astype(fp)
    y = mu[:, 1].astype(fp)
    xc = (x - x.mean()).astype(fp)
    yc = (y - y.mean()).astype(fp)
    det = (a * c - b * b).astype(fp)
    r = np.sqrt(det).astype(fp)
    ir = (fp(1.0) / r).astype(fp)
    one = np.ones(N, fp)
    gN = (np.stack([one, xc, yc, xc * xc, yc * yc, xc * yc, a, b, c,
                    a * yc, a * yc * yc, b * xc, b * yc, b * xc * yc,
                    c * xc, c * xc * xc]) * ir).astype(fp)
    # 0.25 quad scale folded here (device computes exp(-0.5(lnD' + N' rec)))
    fN = (np.stack([0.5 * c * xc * xc + 0.5 * a * yc * yc - b * xc * yc,
                    -c * xc + b * yc,
                    -a * yc + b * xc,
                    0.5 * c, 0.5 * a, -b,
                    0.5 * yc * yc, -xc * yc, 0.5 * xc * xc,
                    -yc, 0.5 * one, yc, xc, -one, -xc, 0.5 * one])
          * (0.25 * ir)).astype(fp)
    gD = np.stack([ir, r, c * ir, a * ir, b * ir]).astype(fp)
    fD = np.stack([0.25 * r, 0.25 * ir, 0.25 * a * ir, 0.25 * c * ir,
                   -0.5 * b * ir]).astype(fp)
    lam = 0.5 * (a + c) + np.sqrt((0.5 * (a - c)) ** 2 + b * b)
    return fN, gN, fD, gD, lam


def _plan(mu, sigma, lam):
    perm = np.argsort(mu[:, 0], kind="stable")
    xo = mu[perm, 0]
    yo = mu[perm, 1]
    lo = lam[perm]

    bb = np.zeros((NCH, 4))
    lmax = np.zeros(NCH)
    for i in range(NCH):
        s = slice(i * CH, (i + 1) * CH)
        bb[i] = [xo[s].min(), xo[s].max(), yo[s].min(), yo[s].max()]
        lmax[i] = lo[s].max()
    dxb = np.maximum(0, np.maximum(bb[:, 0][:, None] - bb[:, 1][None, :],
                                   bb[:, 0][None, :] - bb[:, 1][:, None]))
    dyb = np.maximum(0, np.maximum(bb[:, 2][:, None] - bb[:, 3][None, :],
                                   bb[:, 2][None, :] - bb[:, 3][:, None]))
    d2 = dxb * dxb + dyb * dyb
    lM = 0.5 * (lmax[:, None] + lmax[None, :])
    bnd = np.exp(-0.125 * d2 / lM)

    covered = set()
    for ci in range(NCH):
        for k in range(4):
            cj = (ci + k) % NCH
            covered.add((min(ci, cj), max(ci, cj)))

    extra_pairs = []
    for ci in range(NCH):
        for cj in range(ci + 1, NCH):
            if (ci, cj) in covered:
                continue
            if bnd[ci, cj] > T_A:
                extra_pairs.append((ci, cj))

    from collections import defaultdict
    partners = defaultdict(list)
    for (ci, cj) in extra_pairs:
        partners[ci].append(cj)
    for ci in list(partners):
        while len(partners[ci]) % 4 != 0 and partners[ci]:
            moved = False
            for cj in sorted(partners[ci]):
                if len(partners[cj]) % 4 != 0 and len(partners[cj]) > 0:
                    partners[ci].remove(cj)
                    partners[cj].append(ci)
                    moved = True
                    break
            if not moved:
                break

    def is_b(ci, cj):
        return bnd[min(ci, cj), max(ci, cj)] > BETA * 0.95

    extra_tiles = []
    for ci, ps in sorted(partners.items()):
        # B-relevant partners first so B-tiles pack into the leading groups
        ps = sorted(ps, key=lambda cj: (not is_b(ci, cj), cj))
        for g in range(0, len(ps), 4):
            grp = ps[g:g + 4]
            has_b = any(is_b(ci, cj) for cj in grp)
            grp = grp + [-1] * (4 - len(grp))
            extra_tiles.append((ci, grp, has_b))

    # B-tiles first overall; round-robin deal to cores
    extra_tiles.sort(key=lambda et: not et[2])
    Te = max(1, -(-len(extra_tiles) // N_CORES))
    core_extras = [[] for _ in range(N_CORES)]
    for i, et in enumerate(extra_tiles):
        core_extras[i % N_CORES].append(et)
    TBe = max((sum(1 for et in ce if et[2]) for ce in core_extras), default=0)
    T = Te + 4
    return perm, core_extras, T, Te, TBe


def host_prep(mu, sigma, close_mask):
    fp = np.float32
    fN, gN, fD, gD, lam = _features(mu, sigma)
    perm, core_extras, T, Te, TBe = _plan(mu, sigma, lam)

    fNp = fN[:, perm]
    gNp = gN[:, perm]
    fh_a = fNp.astype(ml_dtypes.bfloat16)
    fl_a = (fNp - fh_a.astype(fp)).astype(ml_dtypes.bfloat16)
    gh_a = gNp.astype(ml_dtypes.bfloat16)
    gl_a = (gNp - gh_a.astype(fp)).astype(ml_dtypes.bfloat16)
    fd_a = fD[:, perm].astype(np.float16)
    gd_a = gD[:, perm].astype(np.float16)
    assert np.isfinite(fd_a.astype(fp)).all() and np.isfinite(gd_a.astype(fp)).all()

    TeX = max(Te, 1)
    in_maps = []
    for k in range(N_CORES):
        ft = np.zeros((16, TeX * 256), ml_dtypes.bfloat16)
        fd = np.zeros((5, TeX * 128), np.float16)
        gt = np.zeros((16, TeX * 1024), ml_dtypes.bfloat16)
        gd = np.zeros((5, TeX * 512), np.float16)
        # default pads for all slots (overwritten by real extras below)
        for t in range(TeX):
            ft[:, t * 256:t * 256 + 128] = fh_a[:, 0:CH]
            ft[:, t * 256 + 128:(t + 1) * 256] = fl_a[:, 0:CH]
            fd[:, t * 128:(t + 1) * 128] = fd_a[:, 0:CH]
            gt[0, t * 1024:t * 1024 + 512] = np.float32(PAD_GH)
            gd[1, t * 512:(t + 1) * 512] = np.float16(PAD_GD)
        for t, (ci, grp, _hb) in enumerate(core_extras[k]):
            rows = slice(ci * CH, (ci + 1) * CH)
            ft[:, t * 256:t * 256 + 128] = fh_a[:, rows]
            ft[:, t * 256 + 128:(t + 1) * 256] = fl_a[:, rows]
            fd[:, t * 128:(t + 1) * 128] = fd_a[:, rows]
            gt[0, t * 1024:t * 1024 + 512] = 0.0
            gd[1, t * 512:(t + 1) * 512] = 0.0
            for s, cj in enumerate(grp):
                gcs = slice(t * 1024 + s * CH, t * 1024 + (s + 1) * CH)
                lcs = slice(t * 1024 + 512 + s * CH, t * 1024 + 512 + (s + 1) * CH)
                dcs = slice(t * 512 + s * CH, t * 512 + (s + 1) * CH)
                if cj >= 0:
                    src = slice(cj * CH, (cj + 1) * CH)
                    gt[:, gcs] = gh_a[:, src]
                    gt[:, lcs] = gl_a[:, src]
                    gd[:, dcs] = gd_a[:, src]
                else:
                    gt[0, gcs] = np.float32(PAD_GH)
                    gd[1, dcs] = np.float16(PAD_GD)

        scol = (4 * k * CH + np.arange(896)) % N
        fcol = slice(4 * k * CH, (4 * k + 4) * CH)
        fs = np.zeros((16, 1024), ml_dtypes.bfloat16)
        fs[:, 0:512] = fh_a[:, fcol]
        fs[:, 512:1024] = fl_a[:, fcol]
        fds = fd_a[:, fcol].copy()
        gs = np.zeros((16, 1792), ml_dtypes.bfloat16)
        gs[:, 0:896] = gh_a[:, scol]
        gs[:, 896:1792] = gl_a[:, scol]
        gds = gd_a[:, scol].copy()
        in_maps.append({"ft": ft, "fd": fd, "gt": gt, "gd": gd,
                        "fs": fs, "fds": fds, "gs": gs, "gds": gds})

    # ---- host-side exact corrections (fp64, sparse) ----
    a64 = sigma[:, 0, 0].astype(np.float64)
    b64 = sigma[:, 0, 1].astype(np.float64)
    c64 = sigma[:, 1, 1].astype(np.float64)
    det64 = a64 * c64 - b64 * b64

    cm = close_mask
    ii, jj = np.nonzero(cm | cm.T)
    sel = ii < jj
    ii, jj = ii[sel], jj[sel]
    w_corr = cm[ii, jj].astype(np.float64) + cm[jj, ii].astype(np.float64)
    aM = 0.5 * (a64[ii] + a64[jj])
    bM = 0.5 * (b64[ii] + b64[jj])
    cM = 0.5 * (c64[ii] + c64[jj])
    detM = aM * cM - bM * bM
    dx = mu[ii, 0].astype(np.float64) - mu[jj, 0]
    dy = mu[ii, 1].astype(np.float64) - mu[jj, 1]
    quad = (cM * dx * dx - 2 * bM * dx * dy + aM * dy * dy) / detM
    t1 = np.sqrt(np.clip(det64[ii] * det64[jj], EPS, None))
    coef = np.exp(-0.125 * quad) * np.sqrt(np.clip(t1 / detM, EPS, None))
    corr_A = float((w_corr * coef).sum())
    corr_B = float((w_corr * np.maximum(coef - BETA, 0.0)).sum())

    half_tr = 0.5 * (a64 + c64)
    disc = np.sqrt((0.5 * (a64 - c64)) ** 2 + b64 * b64)
    eigs = np.stack([half_tr - disc, half_tr + disc], axis=-1)
    L = np.sqrt(np.clip(eigs, EPS, None))
    loss_lamb = float(LAMB * np.log1p(np.abs(L)).mean())

    host = dict(corr_A=corr_A, corr_B=corr_B, loss_lamb=loss_lamb,
                T=T, Te=Te, TBe=TBe)
    return in_maps, host


def kernel(mu, sigma, close_mask):
    mu = np.asarray(mu)
    sigma = np.asarray(sigma)
    close_mask = np.asarray(close_mask)
    in_maps, host = host_prep(mu, sigma, close_mask)
    T, Te, TBe = host["T"], host["Te"], host["TBe"]
    nc = build_kernel(T, Te, TBe)
    res = run_bass_kernel_spmd(nc, in_maps, list(range(N_CORES)))
    A_dev = 0.0
    B_acc = 0.0
    for i in range(N_CORES):
        o = res.results[i]["out"].astype(np.float64)
        A_dev += float(o[:, 0:T].sum())
        B_acc += float(o[:, T:2 * T].sum())
    n_b_tiles = (TBe + 4) * N_CORES
    B_dev = B_acc - BETA * (65536.0 * n_b_tiles)
    A = 2.0 * A_dev - host["corr_A"]
    B = 2.0 * B_dev - host["corr_B"]
    S = ALPHA * A + (1.0 - ALPHA) * B
    total = np.float32(host["loss_lamb"] + S / N)
    return np.asarray(total, dtype=np.float32)
